# revision 40
# baseline (speedup 1.0000x reference)
# GRU decoder kernel for Trainium2 (Bass/Tile), data-parallel over batch.
#
# Problem (per reference):
#   h0 = tanh(latent @ Wd + bd)                      [B, H]
#   x  = latent @ W + b[0]; xz, xr, xh = split(x, 3) [B, 3H]
#   for t in range(T):   (reset_after GRU, recurrent bias b[1])
#       rec = h @ U + b[1]; rz, rr, rh = split(rec, 3)
#       z = sigmoid(xz + rz); r = sigmoid(xr + rr)
#       hh = tanh(xh + r * rh)
#       h = z*h + (1-z)*hh        -> out[:, t, :]
#
# Sharding: batch 1024 -> 8 cores x 128 rows; weights replicated; the T loop
# runs locally per core (no collectives).
#
# Design: fully TRANSPOSED recurrence. Every per-step tensor lives in a
# "blocked-transposed" layout: partition p = feature col within a 128-chunk,
# free axis = [chunk j (4)] x [batch b]. The recurrent matmul is
#   recT[col, b] = sum_k U[k, col] * hT[k, b]
# with U chunks stationary and hT (produced directly in this layout by the
# previous step) moving, all in bf16 (1 cyc/row at any moving size):
#   - no transposes anywhere in the loop (the classic layout needs 4 PE
#     transposes + PSUM->SBUF copies per step, all on the critical path)
#   - matmul cost scales with the moving free size (= batch), so the batch
#     splits into NS=4 independent interleaved streams (32 rows each): each
#     stream's elementwise tail hides under the other streams' bursts
# The constant x-projections/biases are re-folded into PSUM each step by a
# cheap bf16 identity matmul per accumulation-group slice.
#
# Output is written DMA-contiguous in transposed layout [T, 4, 128, 128]
# (bf16) and de-transposed on the host, which is free for the HW timeline.
#
# Techniques:
#  - 4 streams of 32 batch rows: each stream's elementwise tail hides under
#    the other three streams' matmul bursts, and smaller tiles shorten the
#    per-stream chain latency.
#  - The z-gate columns of U / W / biases are NEGATED host-side, so the
#    packed [zc|r] PSUM bank needs ONE sigmoid: sigmoid(ps) gives
#    [1-z | r] directly (zc = sigmoid(-pre_z)). 2 ACT ops per stream.
#  - hnew = h - zc*(h - hh) = z*h + (1-z)*hh, all-bf16 DVE 2x ops; the
#    e = h - hh subtract runs on Pool.
#  - Streams C and D's late tail ops are software-pipelined into the next
#    iteration so per-engine in-order queues match data-availability order.

import numpy as np

B, LD, H, T_DEF = 1024, 256, 512, 128
H3 = 3 * H
NCORES = 8
BS = B // NCORES      # 128 batch rows per core
NS = 4                # streams per core
SB = BS // NS         # 32 batch rows per stream
NCH = H // 128        # 4 feature chunks
BLK = NCH * SB        # 128 = blocked free size of one stream tile
NKL = LD // 128       # 2 k-chunks of the input projection

_BUILD_CACHE = {}


def _build(T):
    import concourse.bass as bass
    import concourse.mybir as mybir
    import concourse.tile as tile
    from concourse import bacc
    from concourse.masks import make_identity

    f32 = mybir.dt.float32
    bf16 = mybir.dt.bfloat16
    AF = mybir.ActivationFunctionType
    OP = mybir.AluOpType

    nc = bacc.Bacc(None, target_bir_lowering=False, debug=False)

    latT_d = nc.dram_tensor("latT", [LD, BS], bf16, kind="ExternalInput")
    w_d = nc.dram_tensor("w", [LD, H3], bf16, kind="ExternalInput")
    wd_d = nc.dram_tensor("wd", [LD, H], bf16, kind="ExternalInput")
    u_d = nc.dram_tensor("u", [H, H3], bf16, kind="ExternalInput")
    bzr_d = nc.dram_tensor("bzr_blk", [128, 2 * BLK], f32, kind="ExternalInput")
    bh_d = nc.dram_tensor("bh_blk", [128, NS * BLK], bf16, kind="ExternalInput")
    b0h_d = nc.dram_tensor("b0h_blk", [128, BLK], f32, kind="ExternalInput")
    bd_d = nc.dram_tensor("bd_blk", [128, BLK], f32, kind="ExternalInput")
    out_d = nc.dram_tensor("out", [T, NS, 128, BLK], bf16, kind="ExternalOutput")

    with tile.TileContext(nc) as tc:
        with (
            tc.tile_pool(name="singles", bufs=1) as singles,
            tc.tile_pool(name="work", bufs=4) as work,
            tc.tile_pool(name="hpool", bufs=4) as hpool,
            tc.tile_pool(name="ps", bufs=1, space="PSUM") as psum,
        ):
            # ---- load constants -------------------------------------------
            u = [singles.tile([128, H3], bf16, tag=f"u{k}", name=f"u{k}")
                 for k in range(4)]
            for k in range(4):
                nc.sync.dma_start(out=u[k], in_=u_d[128 * k:128 * (k + 1), :])
            w = [singles.tile([128, H3], bf16, tag=f"w{k}", name=f"w{k}")
                 for k in range(NKL)]
            for k in range(NKL):
                nc.sync.dma_start(out=w[k], in_=w_d[128 * k:128 * (k + 1), :])
            wd = [singles.tile([128, H], bf16, tag=f"wd{k}", name=f"wd{k}")
                  for k in range(NKL)]
            for k in range(NKL):
                nc.sync.dma_start(out=wd[k], in_=wd_d[128 * k:128 * (k + 1), :])
            lat = [singles.tile([128, BS], bf16, tag=f"lat{k}", name=f"lat{k}")
                   for k in range(NKL)]
            for k in range(NKL):
                nc.sync.dma_start(out=lat[k], in_=latT_d[128 * k:128 * (k + 1), :])
            bzr = singles.tile([128, 2 * BLK], f32, tag="bzr")
            nc.sync.dma_start(out=bzr, in_=bzr_d[:, :])
            bh = singles.tile([128, NS * BLK], bf16, tag="bh")
            nc.sync.dma_start(out=bh, in_=bh_d[:, :])
            b0h = singles.tile([128, BLK], f32, tag="b0h")
            nc.sync.dma_start(out=b0h, in_=b0h_d[:, :])
            bd = singles.tile([128, BLK], f32, tag="bd")
            nc.sync.dma_start(out=bd, in_=bd_d[:, :])

            ident = singles.tile([128, 128], f32, tag="ident")
            make_identity(nc, ident)
            identr = singles.tile([128, 128], bf16, tag="identr")
            nc.scalar.copy(identr, ident)

            # ---- prologue: x-projections and h0, per stream ---------------
            # xzrT[s] = [-(xz + bz) | xr + br] (z-half negated via w/bzr)
            xzrT = [singles.tile([128, 2 * BLK], bf16, tag=f"xzr{s}",
                                 name=f"xzr{s}") for s in range(NS)]
            xhT = [singles.tile([128, BLK], bf16, tag=f"xh{s}", name=f"xh{s}")
                   for s in range(NS)]
            h_bf = [None] * NS

            def proj(ps_tile, cols, s, wt):
                ms = slice(SB * s, SB * (s + 1))
                for j in range(NCH):
                    sl = ps_tile[:, SB * j: SB * (j + 1)]
                    for k in range(NKL):
                        nc.tensor.matmul(
                            sl, wt[k][:, cols + 128 * j: cols + 128 * (j + 1)],
                            lat[k][:, ms], start=(k == 0), stop=(k == NKL - 1))

            for s in range(NS):
                pzr = psum.tile([128, 2 * BLK], f32, tag=f"zr{s}",
                                name=f"pzr{s}")
                proj(pzr[:, 0:BLK], 0, s, w)          # -xz (w negated)
                proj(pzr[:, BLK:2 * BLK], H, s, w)    # xr
                nc.vector.tensor_add(xzrT[s], pzr, bzr)
                pxh = psum.tile([128, BLK], f32, tag=f"hg{s}", name=f"pxh{s}")
                proj(pxh, 2 * H, s, w)
                nc.vector.tensor_add(xhT[s], pxh, b0h)
                ph0 = psum.tile([128, BLK], f32, tag=f"hg{s}", name=f"ph0{s}")
                proj(ph0, 0, s, wd)
                th = work.tile([128, BLK], f32, tag="th", name=f"th{s}")
                nc.vector.tensor_add(th, ph0, bd)
                h_bf[s] = hpool.tile([128, BLK], bf16, tag=f"h{s}",
                                     name=f"h0_{s}")
                nc.scalar.activation(h_bf[s], th, AF.Tanh)

            # ---- steady-state T loop --------------------------------------
            # PSUM banks (bufs=1, 8 total): per stream one packed [zc|r]
            # bank [128, 256] (z-slots emitted first so the r slots close the
            # bank: the combined sigmoid reads it once all groups close) and
            # one h bank [128, 128].
            def mk(s, nm, tt):
                return work.tile([128, BLK], bf16, tag=f"{nm}{s}",
                                 name=f"{nm}{s}_{tt}")

            def emit_burst(s, ps_zr, ps_h, t):
                sls = []
                for j in range(NCH):      # z slots first (negated U cols)
                    sls.append((ps_zr[:, SB * j: SB * (j + 1)], 128 * j,
                                xzrT[s][:, SB * j: SB * (j + 1)]))
                for j in range(NCH):      # r slots close the zr bank
                    sls.append((ps_zr[:, BLK + SB * j: BLK + SB * (j + 1)],
                                H + 128 * j,
                                xzrT[s][:, BLK + SB * j: BLK + SB * (j + 1)]))
                for j in range(NCH):      # h gate, own bank
                    sls.append((ps_h[:, SB * j: SB * (j + 1)],
                                2 * H + 128 * j,
                                bh[:, BLK * s + SB * j: BLK * s + SB * (j + 1)]))
                for sl, base, bias in sls:
                    nc.tensor.matmul(sl, identr, bias, start=True, stop=False)
                    for k in range(4):
                        nc.tensor.matmul(
                            sl, u[k][:, base: base + 128],
                            h_bf[s][:, SB * k: SB * (k + 1)],
                            start=False, stop=(k == 3))

            def emit_sig_t1_t2(s, ps_zr, ps_h, t):
                zcr = work.tile([128, 2 * BLK], bf16, tag=f"zcr{s}",
                                name=f"zcr{s}_{t}")
                nc.scalar.activation(zcr, ps_zr, AF.Sigmoid)
                t1 = mk(s, "t1", t)
                nc.vector.tensor_mul(t1, zcr[:, BLK:2 * BLK], ps_h)
                t2 = mk(s, "t2", t)
                nc.vector.tensor_add(t2, t1, xhT[s])
                return zcr, t2

            def emit_hh(s, t2, t):
                hh = mk(s, "hh", t)
                nc.scalar.activation(hh, t2, AF.Tanh)
                return hh

            def emit_v(s, zcr, hprev, t):
                # off-chain Pool half: v = zc*h, right after the sigmoid
                vv = mk(s, "v", t)
                nc.gpsimd.tensor_mul(vv, zcr[:, 0:BLK], hprev)
                return vv

            def emit_u1(s, vv, hprev, t):
                # DVE half: u1 = h - v (= z*h); emitted where DVE has slack
                u1 = mk(s, "u", t)
                nc.vector.tensor_sub(u1, hprev, vv)
                return u1

            def emit_rest(s, zcr, hh, u1, hnew_t, t):
                # post-tanh: 2 DVE stages, no Pool crossing
                w1 = mk(s, "w", t)
                nc.vector.tensor_mul(w1, zcr[:, 0:BLK], hh)
                nc.vector.tensor_add(hnew_t, u1, w1)
                nc.sync.dma_start(out=out_d[t, s], in_=hnew_t)

            # Streams B and C defer (e, f, hnew, dma) into the next
            # iteration; stream D additionally defers hh. Their deps are
            # satisfied by the time the next iteration starts, so the
            # deferred ops drain immediately without convoying ahead of the
            # next step's chain-head ops on ACT/DVE.
            pend_b = None   # (zcr, hh, hprev, hnew_tile, t)
            pend_c = None   # (zcr, hh, hprev, hnew_tile, t)
            pend_d = None   # (zcr, t2, hprev, hnew_tile, t)

            for t in range(T):
                ps_zr = [psum.tile([128, 2 * BLK], f32, tag=f"zr{s}",
                                   name=f"pszr{s}_{t}") for s in range(NS)]
                ps_h = [psum.tile([128, BLK], f32, tag=f"hg{s}",
                                  name=f"psh{s}_{t}") for s in range(NS)]
                hnew = [hpool.tile([128, BLK], bf16, tag=f"h{s}",
                                   name=f"h{s}_{t}") for s in range(NS)]

                # flush all deferred tails from t-1 at the top: every dep
                # is already satisfied (or nearly), the w1/hnew DVE pairs
                # drain before t1_A's data arrives, and D's hh fits on ACT
                # before sig_A's data closes
                if pend_b is not None:
                    zcrb, hhb, u1b, hnb, tb_ = pend_b
                    emit_rest(1, zcrb, hhb, u1b, hnb, tb_)
                    pend_b = None
                if pend_c is not None:
                    zcrc, hhc, u1c, hnc, tc_ = pend_c
                    emit_rest(2, zcrc, hhc, u1c, hnc, tc_)
                    pend_c = None
                if pend_d is not None:
                    zcrd, t2d, u1d, hnd, td_ = pend_d
                    hhd = emit_hh(3, t2d, td_)
                    emit_rest(3, zcrd, hhd, u1d, hnd, td_)
                    pend_d = None
                # PE bursts (every stream's h(t-1) is now emitted)
                for s in range(NS):
                    emit_burst(s, ps_zr[s], ps_h[s], t)
                # stream A chain head + off-chain u1 (A's inline tail
                # consumes u1_a mid-iteration, so it stays early)
                zcr_a, t2_a = emit_sig_t1_t2(0, ps_zr[0], ps_h[0], t)
                v_a = emit_v(0, zcr_a, h_bf[0], t)
                u1_a = emit_u1(0, v_a, h_bf[0], t)
                # stream B chain head
                zcr_b, t2_b = emit_sig_t1_t2(1, ps_zr[1], ps_h[1], t)
                v_b = emit_v(1, zcr_b, h_bf[1], t)
                # stream A tail (fully inline, 2-stage after tanh)
                hh_a = emit_hh(0, t2_a, t)
                emit_rest(0, zcr_a, hh_a, u1_a, hnew[0], t)
                # stream C chain head
                zcr_c, t2_c = emit_sig_t1_t2(2, ps_zr[2], ps_h[2], t)
                v_c = emit_v(2, zcr_c, h_bf[2], t)
                # stream B: hh inline, rest deferred
                hh_b = emit_hh(1, t2_b, t)
                # stream D chain head
                zcr_d, t2_d = emit_sig_t1_t2(3, ps_zr[3], ps_h[3], t)
                v_d = emit_v(3, zcr_d, h_bf[3], t)
                # stream C: hh inline, rest deferred
                hh_c = emit_hh(2, t2_c, t)
                # B/C/D's u1 DVE halves at the iteration end (DVE idle here;
                # they're only consumed by next iteration's flushes)
                u1_b = emit_u1(1, v_b, h_bf[1], t)
                u1_c = emit_u1(2, v_c, h_bf[2], t)
                u1_d = emit_u1(3, v_d, h_bf[3], t)
                pend_b = (zcr_b, hh_b, u1_b, hnew[1], t)
                pend_c = (zcr_c, hh_c, u1_c, hnew[2], t)
                pend_d = (zcr_d, t2_d, u1_d, hnew[3], t)
                h_bf = hnew

            zcrb, hhb, u1b, hnb, tb_ = pend_b
            emit_rest(1, zcrb, hhb, u1b, hnb, tb_)
            zcrc, hhc, u1c, hnc, tc_ = pend_c
            emit_rest(2, zcrc, hhc, u1c, hnc, tc_)
            zcrd, t2d, u1d, hnd, td_ = pend_d
            hhd = emit_hh(3, t2d, td_)
            emit_rest(3, zcrd, hhd, u1d, hnd, td_)

    nc.compile()
    return nc


def _prep_inputs(latent, Wd, bd, W, U, b):
    import ml_dtypes

    bfd = ml_dtypes.bfloat16
    b0, b1 = b[0], b[1]
    bzr_vec = (b0 + b1)[: 2 * H].copy()
    bzr_vec[:H] *= -1.0                   # negate z constants

    def blk(vec):
        m = vec.reshape(NCH, 128).T       # [128, NCH]
        return np.ascontiguousarray(
            np.repeat(m[:, :, None], SB, axis=2).reshape(128, NCH * SB)
        ).astype(np.float32)

    bzr_blk = np.concatenate([blk(bzr_vec[:H]), blk(bzr_vec[H:])], axis=1)
    bh_one = blk(b1[2 * H:])
    bh_blk = np.concatenate([bh_one] * NS, axis=1)
    b0h_blk = blk(b0[2 * H:])
    bd_blk = blk(bd)
    Wn = W.copy()
    Wn[:, :H] *= -1.0                     # negate z columns
    Un = U.copy()
    Un[:, :H] *= -1.0
    return {
        "w": Wn.astype(bfd), "wd": Wd.astype(bfd), "u": Un.astype(bfd),
        "bzr_blk": bzr_blk, "bh_blk": bh_blk, "b0h_blk": b0h_blk,
        "bd_blk": bd_blk,
    }, bfd


def kernel(latent, Wd, bd, W, U, b, T, _trace=False):
    from concourse.bass_utils import run_bass_kernel_spmd

    latent = np.ascontiguousarray(np.asarray(latent, dtype=np.float32))
    Wd = np.ascontiguousarray(np.asarray(Wd, dtype=np.float32))
    bd = np.ascontiguousarray(np.asarray(bd, dtype=np.float32))
    W = np.ascontiguousarray(np.asarray(W, dtype=np.float32))
    U = np.ascontiguousarray(np.asarray(U, dtype=np.float32))
    b = np.ascontiguousarray(np.asarray(b, dtype=np.float32))
    T = int(T)

    key = (T,)
    if key not in _BUILD_CACHE:
        _BUILD_CACHE[key] = _build(T)
    nc = _BUILD_CACHE[key]

    shared, bfd = _prep_inputs(latent, Wd, bd, W, U, b)

    in_maps = []
    for c in range(NCORES):
        rows = slice(c * BS, (c + 1) * BS)
        m = dict(shared)
        m["latT"] = np.ascontiguousarray(latent[rows].T).astype(bfd)
        in_maps.append(m)

    res = run_bass_kernel_spmd(nc, in_maps, core_ids=list(range(NCORES)),
                               trace=_trace)
    if _trace and res.exec_time_ns is not None:
        print(f"HW exec time: {res.exec_time_ns} ns")

    outs = []
    for c in range(NCORES):
        arr = np.asarray(res.results[c]["out"]).astype(np.float32)
        arr = arr.reshape(T, NS, 128, NCH, SB)
        outs.append(np.transpose(arr, (1, 4, 0, 3, 2)).reshape(BS, T, H))
    return np.concatenate(outs, axis=0)


# revision 41
# speedup vs baseline: 1.0742x; 1.0742x over previous
# GRU decoder kernel for Trainium2 (Bass/Tile), data-parallel over batch.
#
# Problem (per reference):
#   h0 = tanh(latent @ Wd + bd)                      [B, H]
#   x  = latent @ W + b[0]; xz, xr, xh = split(x, 3) [B, 3H]
#   for t in range(T):   (reset_after GRU, recurrent bias b[1])
#       rec = h @ U + b[1]; rz, rr, rh = split(rec, 3)
#       z = sigmoid(xz + rz); r = sigmoid(xr + rr)
#       hh = tanh(xh + r * rh)
#       h = z*h + (1-z)*hh        -> out[:, t, :]
#
# Sharding: batch 1024 -> 8 cores x 128 rows; weights replicated; the T loop
# runs locally per core (no collectives).
#
# Design: fully TRANSPOSED recurrence. Every per-step tensor lives in a
# "blocked-transposed" layout: partition p = feature col within a 128-chunk,
# free axis = [chunk j (4)] x [batch b]. The recurrent matmul is
#   recT[col, b] = sum_k U[k, col] * hT[k, b]
# with U chunks stationary and hT (produced directly in this layout by the
# previous step) moving, all in bf16 (1 cyc/row at any moving size):
#   - no transposes anywhere in the loop (the classic layout needs 4 PE
#     transposes + PSUM->SBUF copies per step, all on the critical path)
#   - matmul cost scales with the moving free size (= batch), so the batch
#     splits into NS=4 independent interleaved streams (32 rows each): each
#     stream's elementwise tail hides under the other streams' bursts
# The constant x-projections/biases are re-folded into PSUM each step by a
# cheap bf16 identity matmul per accumulation-group slice.
#
# Output is written DMA-contiguous in transposed layout [T, 4, 128, 128]
# (bf16) and de-transposed on the host, which is free for the HW timeline.
#
# Techniques:
#  - 4 streams of 32 batch rows: each stream's elementwise tail hides under
#    the other three streams' matmul bursts, and smaller tiles shorten the
#    per-stream chain latency.
#  - The z-gate columns of U / W / biases are NEGATED host-side, so the
#    packed [zc|r] PSUM bank needs ONE sigmoid: sigmoid(ps) gives
#    [1-z | r] directly (zc = sigmoid(-pre_z)). 2 ACT ops per stream.
#  - hnew = h - zc*(h - hh) = z*h + (1-z)*hh, all-bf16 DVE 2x ops; the
#    e = h - hh subtract runs on Pool.
#  - Streams C and D's late tail ops are software-pipelined into the next
#    iteration so per-engine in-order queues match data-availability order.

import numpy as np

B, LD, H, T_DEF = 1024, 256, 512, 128
H3 = 3 * H
NCORES = 8
BS = B // NCORES      # 128 batch rows per core
NS = 4                # streams per core
SB = BS // NS         # 32 batch rows per stream
NCH = H // 128        # 4 feature chunks
BLK = NCH * SB        # 128 = blocked free size of one stream tile
NKL = LD // 128       # 2 k-chunks of the input projection

_BUILD_CACHE = {}


def _build(T):
    import concourse.bass as bass
    import concourse.mybir as mybir
    import concourse.tile as tile
    from concourse import bacc
    from concourse.masks import make_identity

    f32 = mybir.dt.float32
    bf16 = mybir.dt.bfloat16
    AF = mybir.ActivationFunctionType
    OP = mybir.AluOpType

    nc = bacc.Bacc(None, target_bir_lowering=False, debug=False)

    latT_d = nc.dram_tensor("latT", [LD, BS], bf16, kind="ExternalInput")
    w_d = nc.dram_tensor("w", [LD, H3], bf16, kind="ExternalInput")
    wd_d = nc.dram_tensor("wd", [LD, H], bf16, kind="ExternalInput")
    u_d = nc.dram_tensor("u", [H, H3], bf16, kind="ExternalInput")
    bzr_d = nc.dram_tensor("bzr_blk", [128, 2 * BLK], f32, kind="ExternalInput")
    bh_d = nc.dram_tensor("bh_blk", [128, NS * BLK], bf16, kind="ExternalInput")
    b0h_d = nc.dram_tensor("b0h_blk", [128, BLK], f32, kind="ExternalInput")
    bd_d = nc.dram_tensor("bd_blk", [128, BLK], f32, kind="ExternalInput")
    out_d = nc.dram_tensor("out", [T, NS, 128, BLK], bf16, kind="ExternalOutput")

    with tile.TileContext(nc) as tc:
        with (
            tc.tile_pool(name="singles", bufs=1) as singles,
            tc.tile_pool(name="work", bufs=4) as work,
            tc.tile_pool(name="hpool", bufs=4) as hpool,
            tc.tile_pool(name="ps", bufs=1, space="PSUM") as psum,
        ):
            # ---- load constants -------------------------------------------
            u = [singles.tile([128, H3], bf16, tag=f"u{k}", name=f"u{k}")
                 for k in range(4)]
            for k in range(4):
                nc.sync.dma_start(out=u[k], in_=u_d[128 * k:128 * (k + 1), :])
            w = [singles.tile([128, H3], bf16, tag=f"w{k}", name=f"w{k}")
                 for k in range(NKL)]
            for k in range(NKL):
                nc.sync.dma_start(out=w[k], in_=w_d[128 * k:128 * (k + 1), :])
            wd = [singles.tile([128, H], bf16, tag=f"wd{k}", name=f"wd{k}")
                  for k in range(NKL)]
            for k in range(NKL):
                nc.sync.dma_start(out=wd[k], in_=wd_d[128 * k:128 * (k + 1), :])
            lat = [singles.tile([128, BS], bf16, tag=f"lat{k}", name=f"lat{k}")
                   for k in range(NKL)]
            for k in range(NKL):
                nc.sync.dma_start(out=lat[k], in_=latT_d[128 * k:128 * (k + 1), :])
            bzr = singles.tile([128, 2 * BLK], f32, tag="bzr")
            nc.sync.dma_start(out=bzr, in_=bzr_d[:, :])
            bh = singles.tile([128, NS * BLK], bf16, tag="bh")
            nc.sync.dma_start(out=bh, in_=bh_d[:, :])
            b0h = singles.tile([128, BLK], f32, tag="b0h")
            nc.sync.dma_start(out=b0h, in_=b0h_d[:, :])
            bd = singles.tile([128, BLK], f32, tag="bd")
            nc.sync.dma_start(out=bd, in_=bd_d[:, :])

            ident = singles.tile([128, 128], f32, tag="ident")
            make_identity(nc, ident)
            identr = singles.tile([128, 128], bf16, tag="identr")
            nc.scalar.copy(identr, ident)

            # ---- prologue: x-projections and h0, per stream ---------------
            # xzrT[s] = [-(xz + bz) | xr + br] (z-half negated via w/bzr)
            xzrT = [singles.tile([128, 2 * BLK], bf16, tag=f"xzr{s}",
                                 name=f"xzr{s}") for s in range(NS)]
            xhT = [singles.tile([128, BLK], bf16, tag=f"xh{s}", name=f"xh{s}")
                   for s in range(NS)]
            h_bf = [None] * NS

            def proj(ps_tile, cols, s, wt):
                ms = slice(SB * s, SB * (s + 1))
                for j in range(NCH):
                    sl = ps_tile[:, SB * j: SB * (j + 1)]
                    for k in range(NKL):
                        nc.tensor.matmul(
                            sl, wt[k][:, cols + 128 * j: cols + 128 * (j + 1)],
                            lat[k][:, ms], start=(k == 0), stop=(k == NKL - 1))

            for s in range(NS):
                pzr = psum.tile([128, 2 * BLK], f32, tag=f"zr{s}",
                                name=f"pzr{s}")
                proj(pzr[:, 0:BLK], 0, s, w)          # -xz (w negated)
                proj(pzr[:, BLK:2 * BLK], H, s, w)    # xr
                nc.vector.tensor_add(xzrT[s], pzr, bzr)
                pxh = psum.tile([128, BLK], f32, tag=f"hg{s}", name=f"pxh{s}")
                proj(pxh, 2 * H, s, w)
                nc.vector.tensor_add(xhT[s], pxh, b0h)
                ph0 = psum.tile([128, BLK], f32, tag=f"hg{s}", name=f"ph0{s}")
                proj(ph0, 0, s, wd)
                th = work.tile([128, BLK], f32, tag="th", name=f"th{s}")
                nc.vector.tensor_add(th, ph0, bd)
                h_bf[s] = hpool.tile([128, BLK], bf16, tag=f"h{s}",
                                     name=f"h0_{s}")
                nc.scalar.activation(h_bf[s], th, AF.Tanh)

            # ---- steady-state T loop --------------------------------------
            # PSUM banks (bufs=1, 8 total): per stream one packed [zc|r]
            # bank [128, 256] (z-slots emitted first so the r slots close the
            # bank: the combined sigmoid reads it once all groups close) and
            # one h bank [128, 128].
            def mk(s, nm, tt):
                return work.tile([128, BLK], bf16, tag=f"{nm}{s}",
                                 name=f"{nm}{s}_{tt}")

            def emit_burst(s, ps_zr, ps_h, t):
                sls = []
                for j in range(NCH):      # z slots first (negated U cols)
                    sls.append((ps_zr[:, SB * j: SB * (j + 1)], 128 * j,
                                xzrT[s][:, SB * j: SB * (j + 1)]))
                for j in range(NCH):      # r slots close the zr bank
                    sls.append((ps_zr[:, BLK + SB * j: BLK + SB * (j + 1)],
                                H + 128 * j,
                                xzrT[s][:, BLK + SB * j: BLK + SB * (j + 1)]))
                for j in range(NCH):      # h gate, own bank
                    sls.append((ps_h[:, SB * j: SB * (j + 1)],
                                2 * H + 128 * j,
                                bh[:, BLK * s + SB * j: BLK * s + SB * (j + 1)]))
                for sl, base, bias in sls:
                    nc.tensor.matmul(sl, identr, bias, start=True, stop=False)
                    for k in range(4):
                        nc.tensor.matmul(
                            sl, u[k][:, base: base + 128],
                            h_bf[s][:, SB * k: SB * (k + 1)],
                            start=False, stop=(k == 3))

            def emit_sig_t1_t2(s, ps_zr, ps_h, t):
                zcr = work.tile([128, 2 * BLK], bf16, tag=f"zcr{s}",
                                name=f"zcr{s}_{t}")
                nc.scalar.activation(zcr, ps_zr, AF.Sigmoid)
                t1 = mk(s, "t1", t)
                nc.vector.tensor_mul(t1, zcr[:, BLK:2 * BLK], ps_h)
                t2 = mk(s, "t2", t)
                nc.vector.tensor_add(t2, t1, xhT[s])
                return zcr, t2

            def emit_hh(s, t2, t):
                hh = mk(s, "hh", t)
                nc.scalar.activation(hh, t2, AF.Tanh)
                return hh

            def emit_u1(s, zcr, hprev, t):
                vv = mk(s, "v", t)
                nc.gpsimd.tensor_mul(vv, zcr[:, 0:BLK], hprev)
                u1 = mk(s, "u", t)
                nc.vector.tensor_sub(u1, hprev, vv)
                return u1

            def emit_rest(s, zcr, hh, hprev, hnew_t, t):
                ee = mk(s, "e", t)
                nc.gpsimd.tensor_sub(ee, hprev, hh)
                ff = mk(s, "f", t)
                nc.vector.tensor_mul(ff, zcr[:, 0:BLK], ee)
                nc.vector.tensor_sub(hnew_t, hprev, ff)
                nc.sync.dma_start(out=out_d[t, s], in_=hnew_t)

            # Streams B and C defer (e, f, hnew, dma) into the next
            # iteration; stream D additionally defers hh. Their deps are
            # satisfied by the time the next iteration starts, so the
            # deferred ops drain immediately without convoying ahead of the
            # next step's chain-head ops on ACT/DVE.
            pend_b = None   # (zcr, hh, hprev, hnew_tile, t)
            pend_c = None   # (zcr, hh, hprev, hnew_tile, t)
            pend_d = None   # (zcr, t2, hprev, hnew_tile, t)

            for t in range(T):
                ps_zr = [psum.tile([128, 2 * BLK], f32, tag=f"zr{s}",
                                   name=f"pszr{s}_{t}") for s in range(NS)]
                ps_h = [psum.tile([128, BLK], f32, tag=f"hg{s}",
                                  name=f"psh{s}_{t}") for s in range(NS)]
                hnew = [hpool.tile([128, BLK], bf16, tag=f"h{s}",
                                   name=f"h{s}_{t}") for s in range(NS)]

                # flush streams B and C's late tails from t-1 (deps ready)
                if pend_b is not None:
                    zcrb, hhb, hpb, hnb, tb_ = pend_b
                    emit_rest(1, zcrb, hhb, hpb, hnb, tb_)
                    pend_b = None
                if pend_c is not None:
                    zcrc, hhc, hpc, hnc, tc_ = pend_c
                    emit_rest(2, zcrc, hhc, hpc, hnc, tc_)
                    pend_c = None
                # stream D's hh/e from t-1: hh on ACT fits before sig_A's
                # data closes; f/hnew stay after A's chain head
                d_mid = None
                if pend_d is not None:
                    zcrd, t2d, hpd, hnd, td_ = pend_d
                    hhd = emit_hh(3, t2d, td_)
                    eed = mk(3, "e", td_)
                    nc.gpsimd.tensor_sub(eed, hpd, hhd)
                    d_mid = (zcrd, eed, hpd, hnd, td_)
                    pend_d = None
                # PE bursts A, B, C (their h(t-1) is complete)
                for s in range(3):
                    emit_burst(s, ps_zr[s], ps_h[s], t)
                # stream A chain head + off-chain u1 = h - zc*h
                zcr_a, t2_a = emit_sig_t1_t2(0, ps_zr[0], ps_h[0], t)
                u1_a = emit_u1(0, zcr_a, h_bf[0], t)
                # finish stream D's tail from t-1, then its burst
                if d_mid is not None:
                    zcrd, eed, hpd, hnd, td_ = d_mid
                    ffd = mk(3, "f", td_)
                    nc.vector.tensor_mul(ffd, zcrd[:, 0:BLK], eed)
                    nc.vector.tensor_sub(hnd, hpd, ffd)
                    nc.sync.dma_start(out=out_d[td_, 3], in_=hnd)
                emit_burst(3, ps_zr[3], ps_h[3], t)
                # stream B chain head
                zcr_b, t2_b = emit_sig_t1_t2(1, ps_zr[1], ps_h[1], t)
                # stream A tail (inline, 2-stage after tanh)
                hh_a = emit_hh(0, t2_a, t)
                w1_a = mk(0, "w", t)
                nc.vector.tensor_mul(w1_a, zcr_a[:, 0:BLK], hh_a)
                nc.vector.tensor_add(hnew[0], u1_a, w1_a)
                nc.sync.dma_start(out=out_d[t, 0], in_=hnew[0])
                # stream C chain head
                zcr_c, t2_c = emit_sig_t1_t2(2, ps_zr[2], ps_h[2], t)
                # stream B: hh inline, rest deferred
                hh_b = emit_hh(1, t2_b, t)
                pend_b = (zcr_b, hh_b, h_bf[1], hnew[1], t)
                # stream D chain head
                zcr_d, t2_d = emit_sig_t1_t2(3, ps_zr[3], ps_h[3], t)
                # stream C: hh inline, rest deferred
                hh_c = emit_hh(2, t2_c, t)
                pend_c = (zcr_c, hh_c, h_bf[2], hnew[2], t)
                pend_d = (zcr_d, t2_d, h_bf[3], hnew[3], t)
                h_bf = hnew

            zcrb, hhb, hpb, hnb, tb_ = pend_b
            emit_rest(1, zcrb, hhb, hpb, hnb, tb_)
            zcrc, hhc, hpc, hnc, tc_ = pend_c
            emit_rest(2, zcrc, hhc, hpc, hnc, tc_)
            zcrd, t2d, hpd, hnd, td_ = pend_d
            hhd = emit_hh(3, t2d, td_)
            emit_rest(3, zcrd, hhd, hpd, hnd, td_)

    nc.compile()
    return nc


def _prep_inputs(latent, Wd, bd, W, U, b):
    import ml_dtypes

    bfd = ml_dtypes.bfloat16
    b0, b1 = b[0], b[1]
    bzr_vec = (b0 + b1)[: 2 * H].copy()
    bzr_vec[:H] *= -1.0                   # negate z constants

    def blk(vec):
        m = vec.reshape(NCH, 128).T       # [128, NCH]
        return np.ascontiguousarray(
            np.repeat(m[:, :, None], SB, axis=2).reshape(128, NCH * SB)
        ).astype(np.float32)

    bzr_blk = np.concatenate([blk(bzr_vec[:H]), blk(bzr_vec[H:])], axis=1)
    bh_one = blk(b1[2 * H:])
    bh_blk = np.concatenate([bh_one] * NS, axis=1)
    b0h_blk = blk(b0[2 * H:])
    bd_blk = blk(bd)
    Wn = W.copy()
    Wn[:, :H] *= -1.0                     # negate z columns
    Un = U.copy()
    Un[:, :H] *= -1.0
    return {
        "w": Wn.astype(bfd), "wd": Wd.astype(bfd), "u": Un.astype(bfd),
        "bzr_blk": bzr_blk, "bh_blk": bh_blk, "b0h_blk": b0h_blk,
        "bd_blk": bd_blk,
    }, bfd


def kernel(latent, Wd, bd, W, U, b, T, _trace=False):
    from concourse.bass_utils import run_bass_kernel_spmd

    latent = np.ascontiguousarray(np.asarray(latent, dtype=np.float32))
    Wd = np.ascontiguousarray(np.asarray(Wd, dtype=np.float32))
    bd = np.ascontiguousarray(np.asarray(bd, dtype=np.float32))
    W = np.ascontiguousarray(np.asarray(W, dtype=np.float32))
    U = np.ascontiguousarray(np.asarray(U, dtype=np.float32))
    b = np.ascontiguousarray(np.asarray(b, dtype=np.float32))
    T = int(T)

    key = (T,)
    if key not in _BUILD_CACHE:
        _BUILD_CACHE[key] = _build(T)
    nc = _BUILD_CACHE[key]

    shared, bfd = _prep_inputs(latent, Wd, bd, W, U, b)

    in_maps = []
    for c in range(NCORES):
        rows = slice(c * BS, (c + 1) * BS)
        m = dict(shared)
        m["latT"] = np.ascontiguousarray(latent[rows].T).astype(bfd)
        in_maps.append(m)

    res = run_bass_kernel_spmd(nc, in_maps, core_ids=list(range(NCORES)),
                               trace=_trace)
    if _trace and res.exec_time_ns is not None:
        print(f"HW exec time: {res.exec_time_ns} ns")

    outs = []
    for c in range(NCORES):
        arr = np.asarray(res.results[c]["out"]).astype(np.float32)
        arr = arr.reshape(T, NS, 128, NCH, SB)
        outs.append(np.transpose(arr, (1, 4, 0, 3, 2)).reshape(BS, T, H))
    return np.concatenate(outs, axis=0)
